# revision 17
# baseline (speedup 1.0000x reference)
"""Distributed self-attention kernel for Trainium2, 8 NeuronCores.

Head-parallel sharding: NH=16 heads across 8 cores = one even/odd head
pair per core. Each core computes q/k/v projections for ITS pair over
the FULL sequence from the full hidden states (replicated; the 6.3 MB
x^T load streams in 512-column blocks and overlaps the projection
matmuls), runs attention for its 2 heads over all 3072 queries x 3072
keys, and writes its [3072, 128] slice of the hidden dim. No
collectives at all.

Pipeline notes:
  - x^T DMA triggers issue from the Pool sequencer (cheap dispatch),
    emitted before everything else.
  - No bias matmuls: bq/bk are folded into the PSUM->SBUF copies; bv is
    added on the host (ctx/denom + bv is exact since sum_k p_k = 1).
  - Scores in transposed layout (s^T[key, query]): stationary = k^T
    pair-block [128 dims, 128 keys], query rhs zero-padded per head
    ([q_even; 0] / [0; q_odd]) so each head streams at full PE rate.
  - exp on ScalarE with the 1/sqrt(64) scale fused (no max subtraction:
    logits are small; mathematically identical to the reference).
  - Warm start: the projection PSUM pool is slimmed to 2 banks (pq and
    pk share one; the pv matmuls between them drain pq's copies) so the
    score pools coexist with it, and query block 0's score+exp groups
    are interleaved between the cb3/cb5 projections -- ScalarE's
    ~147us exp rail starts ~20us earlier. Its P@V is deferred until the
    projection pool closes and donates its banks to the ctx pool.
  - P@V uses exp'd score tiles as the STATIONARY operand and v columns
    as the moving operand: out accumulates directly in [query, 65]
    layout (64 ctx dims + the softmax denominator from the interleaved
    ones column), so no PE transposes and no PSUM->SBUF ctx copies are
    needed. All six [128q, 65] accumulators of a query block live in
    ONE PSUM bank: only the first matmul of the block carries
    start=True (the hardware clears has_written bank-wide), every later
    matmul accumulates-or-overwrites per element; only the last carries
    stop=True. ctx banks double-buffer across query blocks so the
    VectorE normalize epilogue of block qb overlaps block qb+1.
"""

import numpy as np
import ml_dtypes

import concourse.bacc as bacc
import concourse.mybir as mybir
import concourse.tile as tile
from concourse import bass_utils

F32 = mybir.dt.float32
BF16 = mybir.dt.bfloat16
AF = mybir.ActivationFunctionType

N_CORES = 8
B, S, HID = 1, 3072, 1024
NH, HD = 16, 64
KT = S // 128               # 24 key tiles
CB = 6                      # x streamed in 6 blocks of 512 columns
QB = 8                      # 8 query blocks of 384
QW = S // QB                # 384 queries per block
NG = KT // 3                # 8 groups of 3 key tiles for batched exp
VTW = 208                   # per-kt v stride: [v_e(64) | 1 | v_o(64) | 1 | pad]

_cache: dict = {}


def _build(with_mask: bool):
    nc = bacc.Bacc("TRN2", target_bir_lowering=False, debug=False,
                   num_devices=N_CORES)

    xt = nc.dram_tensor("xt", [HID, S], BF16, kind="ExternalInput")
    w = nc.dram_tensor("w", [3, HID, 128], BF16, kind="ExternalInput")
    bcol = nc.dram_tensor("bcol", [128, 2], F32, kind="ExternalInput")
    if with_mask:
        maskt = nc.dram_tensor("maskt", [128, KT], F32, kind="ExternalInput")
    out = nc.dram_tensor("out", [S, 128], F32, kind="ExternalOutput")

    with tile.TileContext(nc) as tc:
        with tc.tile_pool(name="persist", bufs=1) as pp:
            # ---- persistent SBUF tensors ----
            xsb = pp.tile([128, 8 * S], BF16, tag="xsb")
            qz = pp.tile([128, 2 * S], BF16, tag="qz")
            ksb = pp.tile([128, S], BF16, tag="ksb")
            vsb = pp.tile([128, KT * VTW], BF16, tag="vsb")
            bsb = pp.tile([128, 2], F32, tag="bsb")
            osb = pp.tile([128, KT * 128], F32, tag="osb")
            if with_mask:
                msb = pp.tile([128, KT], F32, tag="msb")

            # x^T streams first, via the cheap Pool sequencer (not the
            # Scalar queue -- it must stay free for the PSUM->SBUF copies)
            def xdma(cb, halves=1):
                c0 = cb * 512
                for j in range(8):
                    wdt = 512 // halves
                    for hh in range(halves):
                        nc.gpsimd.dma_start(
                            xsb[:, j * S + c0 + hh * wdt:
                                j * S + c0 + (hh + 1) * wdt],
                            xt[j * 128:(j + 1) * 128,
                               c0 + hh * wdt:c0 + (hh + 1) * wdt])

            xdma(0, halves=2)
            wt = [[None] * 8 for _ in range(3)]
            for proj in range(3):
                for j in range(8):
                    wt[proj][j] = nc_w = pp.tile([128, 128], BF16,
                                                 tag=f"w{proj}_{j}",
                                                 name=f"w{proj}_{j}")
                    nc.sync.dma_start(nc_w[:],
                                      w[proj, j * 128:(j + 1) * 128, :])
            for cb in range(1, CB):
                xdma(cb)
            vsb3 = vsb.rearrange("p (k y) -> p k y", y=VTW)
            nc.gpsimd.memset(vsb3[:, :, 64:65], 1.0)
            nc.gpsimd.memset(vsb3[:, :, 129:130], 1.0)

            nc.vector.memset(qz[64:128, 0:S], 0.0)
            nc.vector.memset(qz[0:64, S:2 * S], 0.0)

            nc.sync.dma_start(bsb[:], bcol[:])
            if with_mask:
                nc.sync.dma_start(msb[:], maskt[:])

            # ---- phases A+C ----
            with (
                tc.tile_pool(name="spoolE", bufs=1, space="PSUM") as spoolE,
                tc.tile_pool(name="spoolO", bufs=1, space="PSUM") as spoolO,
                tc.tile_pool(name="ppool", bufs=18) as ppool,
                tc.tile_pool(name="rpool", bufs=8) as rpool,
            ):
                def score_block(sp, e, g, q0):
                    for j in range(3):
                        kt = g * 3 + j
                        nc.tensor.matmul(
                            sp[:, j * 512:j * 512 + QW],
                            ksb[:, kt * 128:(kt + 1) * 128],
                            qz[:, e * S + q0:e * S + q0 + QW],
                            start=True, stop=True)

                def exp_block(pt, sp, g):
                    src3 = sp.rearrange("p (g x) -> p g x", x=512)[:, :, 0:QW]
                    dst3 = pt.rearrange("p (g x) -> p g x", x=QW)
                    if with_mask:
                        for j in range(3):
                            kt = g * 3 + j
                            nc.scalar.activation(
                                dst3[:, j, :], src3[:, j, :], AF.Exp,
                                bias=msb[:, kt:kt + 1], scale=0.125)
                    else:
                        nc.scalar.activation(dst3, src3, AF.Exp, scale=0.125)

                def attn_group(qb, g):
                    q0 = qb * QW
                    spE = spoolE.tile([128, 1536], F32, tag="spE",
                                      name=f"spE{qb}_{g}")
                    score_block(spE, 0, g, q0)
                    ptE = ppool.tile([128, 3 * QW], BF16, tag="pt",
                                     name=f"ptE{qb}_{g}")
                    exp_block(ptE, spE, g)
                    spO = spoolO.tile([128, 1536], F32, tag="spO",
                                      name=f"spO{qb}_{g}")
                    score_block(spO, 1, g, q0)
                    ptO = ppool.tile([128, 3 * QW], BF16, tag="pt",
                                     name=f"ptO{qb}_{g}")
                    exp_block(ptO, spO, g)
                    return ptE, ptO

                def pv_block(cx, pt, h, g):
                    # stationary = exp'd scores [128 keys, 128 q], moving =
                    # v window [128 keys, 65]; out accumulates [q, 65] in
                    # region (h, t3) of the block's single ctx bank.
                    for j in range(3):
                        kt = g * 3 + j
                        for t3 in range(3):
                            r = h * 3 + t3
                            nc.tensor.matmul(
                                cx[:, r * 65:(r + 1) * 65],
                                pt[:, j * QW + t3 * 128:
                                   j * QW + (t3 + 1) * 128],
                                vsb[:, kt * VTW + 65 * h:
                                    kt * VTW + 65 * h + 65],
                                start=(g == 0 and j == 0 and h == 0
                                       and t3 == 0),
                                stop=(g == NG - 1 and j == 2 and h == 1
                                      and t3 == 2),
                                skip_group_check=True)

                def epilogue(qb, cx):
                    for t3 in range(3):
                        t = qb * 3 + t3
                        for h in range(2):
                            r = h * 3 + t3
                            rec = rpool.tile([128, 1], F32, tag="rec",
                                             name=f"rec{qb}_{r}")
                            nc.vector.reciprocal(
                                rec[:], cx[:, r * 65 + 64:r * 65 + 65])
                            nc.vector.tensor_scalar_mul(
                                osb[:, t * 128 + h * 64:t * 128 + h * 64 + 64],
                                cx[:, r * 65:r * 65 + 64], rec[:])
                        # split across partitions: two queues drain in parallel
                        nc.sync.dma_start(
                            out[t * 128:t * 128 + 64, :],
                            osb[0:64, t * 128:(t + 1) * 128])
                        nc.sync.dma_start(
                            out[t * 128 + 64:(t + 1) * 128, :],
                            osb[64:128, t * 128:(t + 1) * 128])

                early = []
                # projections: pq and pk share one PSUM bank -- the pv
                # matmuls emitted between them cover pq's copy drain
                with tc.tile_pool(name="ppsum", bufs=1,
                                  space="PSUM") as ppsum:
                    def proj(cb):
                        c0 = cb * 512
                        pq = ppsum.tile([128, 512], F32, tag="pqk",
                                        name=f"pq{cb}")
                        for j in range(8):
                            nc.tensor.matmul(
                                pq[:], wt[0][j][:],
                                xsb[:, j * S + c0:j * S + c0 + 512],
                                start=(j == 0), stop=(j == 7))
                        nc.vector.tensor_scalar_add(
                            qz[0:64, c0:c0 + 512], pq[0:64, :],
                            bsb[0:64, 0:1])
                        nc.vector.tensor_scalar_add(
                            qz[64:128, S + c0:S + c0 + 512], pq[64:128, :],
                            bsb[64:128, 0:1])
                        pv = ppsum.tile([128, 512], F32, tag="pvv",
                                        name=f"pv{cb}")
                        for kk in range(4):
                            for j in range(8):
                                nc.tensor.matmul(
                                    pv[:, kk * 128:(kk + 1) * 128],
                                    xsb[:, j * S + c0 + kk * 128:
                                        j * S + c0 + (kk + 1) * 128],
                                    wt[2][j][:],
                                    start=(j == 0), stop=(j == 7))
                        pv3 = pv.rearrange("p (k y) -> p k y", y=128)
                        vd = vsb3[:, cb * 4:(cb + 1) * 4, :]
                        nc.scalar.copy(vd[:, :, 0:64], pv3[:, :, 0:64])
                        nc.scalar.copy(vd[:, :, 65:129], pv3[:, :, 64:128])
                        pk = ppsum.tile([128, 512], F32, tag="pqk",
                                        name=f"pk{cb}")
                        for j in range(8):
                            nc.tensor.matmul(
                                pk[:], wt[1][j][:],
                                xsb[:, j * S + c0:j * S + c0 + 512],
                                start=(j == 0), stop=(j == 7))
                        nc.scalar.activation(ksb[:, c0:c0 + 512], pk[:],
                                             AF.Identity, bias=bsb[:, 1:2])

                    for cb in range(3):
                        proj(cb)
                    for cb, gs in ((3, (0, 1)), (4, (2, 3)), (5, (4, 5))):
                        proj(cb)
                        for g in gs:
                            early.append((g,) + attn_group(0, g))

                with tc.tile_pool(name="cpsum", bufs=2,
                                  space="PSUM") as cpsum:
                    # finish query block 0: last two groups, deferred P@V
                    cx = cpsum.tile([128, 512], F32, tag="ctx", name="cx0")
                    for g in (6, 7):
                        early.append((g,) + attn_group(0, g))
                    for g, ptE, ptO in early:
                        pv_block(cx, ptE, 0, g)
                        pv_block(cx, ptO, 1, g)
                    epilogue(0, cx)

                    for qb in range(1, QB):
                        cx = cpsum.tile([128, 512], F32, tag="ctx",
                                        name=f"cx{qb}")
                        for g in range(NG):
                            ptE, ptO = attn_group(qb, g)
                            pv_block(cx, ptE, 0, g)
                            pv_block(cx, ptO, 1, g)
                        epilogue(qb, cx)

    nc.compile()
    return nc


def _get_program(with_mask: bool):
    key = ("prog", with_mask)
    if key not in _cache:
        _cache[key] = _build(with_mask)
    return _cache[key]


def kernel(hidden_states, attention_mask, Wq, bq, Wk, bk, Wv, bv):
    x = np.asarray(hidden_states, np.float32).reshape(S, HID)
    mask = np.asarray(attention_mask, np.float32).reshape(-1)
    if mask.size == 1:
        mask = np.full(S, float(mask[0]), np.float32)
    with_mask = bool(np.any(mask))

    # transposed weights [3, 1024, 1024]; biases ride separately
    w_all = np.stack([np.asarray(Wq, np.float32).T,
                      np.asarray(Wk, np.float32).T,
                      np.asarray(Wv, np.float32).T]).astype(ml_dtypes.bfloat16)
    bq = np.asarray(bq, np.float32)
    bk = np.asarray(bk, np.float32)
    bv = np.asarray(bv, np.float32)

    xtc = np.ascontiguousarray(x.T).astype(ml_dtypes.bfloat16)
    if with_mask:
        maskt = np.ascontiguousarray(
            mask.reshape(KT, 128).T.astype(np.float32))

    nc = _get_program(with_mask)
    in_maps = []
    for c in range(N_CORES):
        sl = slice(c * 128, (c + 1) * 128)
        m = {
            "xt": xtc,
            "w": np.ascontiguousarray(w_all[:, :, sl]),
            "bcol": np.ascontiguousarray(
                np.stack([bq[sl], bk[sl]], axis=1)),
        }
        if with_mask:
            m["maskt"] = maskt
        in_maps.append(m)

    _cache["last_in_maps"] = in_maps
    res = bass_utils.run_bass_kernel_spmd(nc, in_maps, core_ids=list(range(N_CORES)))
    out = np.concatenate([res.results[c]["out"] for c in range(N_CORES)], axis=1)
    out = out + bv[None, :]
    return out.reshape(B, S, HID).astype(np.float32)


# revision 19
# speedup vs baseline: 1.0019x; 1.0019x over previous
"""Distributed self-attention kernel for Trainium2, 8 NeuronCores.

Head-parallel sharding: NH=16 heads across 8 cores = one even/odd head
pair per core. Each core computes q/k/v projections for ITS pair over
the FULL sequence from the full hidden states (replicated; the 6.3 MB
x^T load streams in 512-column blocks and overlaps the projection
matmuls), runs attention for its 2 heads over all 3072 queries x 3072
keys, and writes its [3072, 128] slice of the hidden dim. No
collectives at all.

Pipeline notes:
  - x^T DMA triggers issue from the Pool sequencer (cheap dispatch),
    emitted before everything else.
  - No bias matmuls: bq/bk are folded into the PSUM->SBUF copies; bv is
    added on the host (ctx/denom + bv is exact since sum_k p_k = 1).
  - Scores in transposed layout (s^T[key, query]): stationary = k^T
    pair-block [128 dims, 128 keys], query rhs zero-padded per head
    ([q_even; 0] / [0; q_odd]) so each head streams at full PE rate.
  - exp on ScalarE with the 1/sqrt(64) scale fused (no max subtraction:
    logits are small; mathematically identical to the reference).
  - Warm start: the projection PSUM pool is slimmed to 2 banks (pq and
    pk share one; the pv matmuls between them drain pq's copies) so the
    score pools coexist with it, and query block 0's score+exp groups
    are interleaved between the cb3/cb5 projections -- ScalarE's
    ~147us exp rail starts ~20us earlier. Its P@V is deferred until the
    projection pool closes and donates its banks to the ctx pool.
  - P@V uses exp'd score tiles as the STATIONARY operand and v columns
    as the moving operand: out accumulates directly in [query, 65]
    layout (64 ctx dims + the softmax denominator from the interleaved
    ones column), so no PE transposes and no PSUM->SBUF ctx copies are
    needed. All six [128q, 65] accumulators of a query block live in
    ONE PSUM bank: only the first matmul of the block carries
    start=True (the hardware clears has_written bank-wide), every later
    matmul accumulates-or-overwrites per element; only the last carries
    stop=True. ctx banks double-buffer across query blocks so the
    VectorE normalize epilogue of block qb overlaps block qb+1.
"""

import numpy as np
import ml_dtypes

import concourse.bacc as bacc
import concourse.mybir as mybir
import concourse.tile as tile
from concourse import bass_utils

F32 = mybir.dt.float32
BF16 = mybir.dt.bfloat16
AF = mybir.ActivationFunctionType

N_CORES = 8
B, S, HID = 1, 3072, 1024
NH, HD = 16, 64
KT = S // 128               # 24 key tiles
CB = 6                      # x streamed in 6 blocks of 512 columns
QB = 8                      # 8 query blocks of 384
QW = S // QB                # 384 queries per block
NG = KT // 3                # 8 groups of 3 key tiles for batched exp
VTW = 208                   # per-kt v stride: [v_e(64) | 1 | v_o(64) | 1 | pad]

_cache: dict = {}


def _build(with_mask: bool):
    nc = bacc.Bacc("TRN2", target_bir_lowering=False, debug=False,
                   num_devices=N_CORES)

    xt = nc.dram_tensor("xt", [HID, S], BF16, kind="ExternalInput")
    w = nc.dram_tensor("w", [3, HID, 128], BF16, kind="ExternalInput")
    bcol = nc.dram_tensor("bcol", [128, 2], F32, kind="ExternalInput")
    if with_mask:
        maskt = nc.dram_tensor("maskt", [128, KT], F32, kind="ExternalInput")
    out = nc.dram_tensor("out", [S, 128], F32, kind="ExternalOutput")

    with tile.TileContext(nc) as tc:
        with tc.tile_pool(name="persist", bufs=1) as pp:
            # ---- persistent SBUF tensors ----
            xsb = pp.tile([128, 8 * S], BF16, tag="xsb")
            qz = pp.tile([128, 2 * S], BF16, tag="qz")
            ksb = pp.tile([128, S], BF16, tag="ksb")
            vsb = pp.tile([128, KT * VTW], BF16, tag="vsb")
            bsb = pp.tile([128, 2], F32, tag="bsb")
            osb = pp.tile([128, KT * 128], F32, tag="osb")
            if with_mask:
                msb = pp.tile([128, KT], F32, tag="msb")

            # x^T streams first, via the cheap Pool sequencer (not the
            # Scalar queue -- it must stay free for the PSUM->SBUF copies)
            def xdma(cb, halves=1):
                c0 = cb * 512
                for j in range(8):
                    wdt = 512 // halves
                    for hh in range(halves):
                        nc.gpsimd.dma_start(
                            xsb[:, j * S + c0 + hh * wdt:
                                j * S + c0 + (hh + 1) * wdt],
                            xt[j * 128:(j + 1) * 128,
                               c0 + hh * wdt:c0 + (hh + 1) * wdt])

            xdma(0, halves=2)
            wt = [[None] * 8 for _ in range(3)]
            for proj in range(3):
                for j in range(8):
                    wt[proj][j] = nc_w = pp.tile([128, 128], BF16,
                                                 tag=f"w{proj}_{j}",
                                                 name=f"w{proj}_{j}")
                    nc.sync.dma_start(nc_w[:],
                                      w[proj, j * 128:(j + 1) * 128, :])
            for cb in range(1, CB):
                xdma(cb)
            vsb3 = vsb.rearrange("p (k y) -> p k y", y=VTW)
            nc.gpsimd.memset(vsb3[:, :, 64:65], 1.0)
            nc.gpsimd.memset(vsb3[:, :, 129:130], 1.0)

            nc.vector.memset(qz[64:128, 0:S], 0.0)
            nc.vector.memset(qz[0:64, S:2 * S], 0.0)

            nc.sync.dma_start(bsb[:], bcol[:])
            if with_mask:
                nc.sync.dma_start(msb[:], maskt[:])

            # ---- phases A+C ----
            with (
                tc.tile_pool(name="spoolE", bufs=1, space="PSUM") as spoolE,
                tc.tile_pool(name="spoolO", bufs=1, space="PSUM") as spoolO,
                tc.tile_pool(name="ppool", bufs=26) as ppool,
                tc.tile_pool(name="rpool", bufs=8) as rpool,
            ):
                def score_block(sp, e, g, q0):
                    for j in range(3):
                        kt = g * 3 + j
                        nc.tensor.matmul(
                            sp[:, j * 512:j * 512 + QW],
                            ksb[:, kt * 128:(kt + 1) * 128],
                            qz[:, e * S + q0:e * S + q0 + QW],
                            start=True, stop=True)

                def exp_block(pt, sp, g):
                    src3 = sp.rearrange("p (g x) -> p g x", x=512)[:, :, 0:QW]
                    dst3 = pt.rearrange("p (g x) -> p g x", x=QW)
                    if with_mask:
                        for j in range(3):
                            kt = g * 3 + j
                            nc.scalar.activation(
                                dst3[:, j, :], src3[:, j, :], AF.Exp,
                                bias=msb[:, kt:kt + 1], scale=0.125)
                    else:
                        nc.scalar.activation(dst3, src3, AF.Exp, scale=0.125)

                def attn_group(qb, g):
                    q0 = qb * QW
                    spE = spoolE.tile([128, 1536], F32, tag="spE",
                                      name=f"spE{qb}_{g}")
                    score_block(spE, 0, g, q0)
                    ptE = ppool.tile([128, 3 * QW], BF16, tag="pt",
                                     name=f"ptE{qb}_{g}")
                    exp_block(ptE, spE, g)
                    spO = spoolO.tile([128, 1536], F32, tag="spO",
                                      name=f"spO{qb}_{g}")
                    score_block(spO, 1, g, q0)
                    ptO = ppool.tile([128, 3 * QW], BF16, tag="pt",
                                     name=f"ptO{qb}_{g}")
                    exp_block(ptO, spO, g)
                    return ptE, ptO

                def pv_block(cx, pt, h, g):
                    # stationary = exp'd scores [128 keys, 128 q], moving =
                    # v window [128 keys, 65]; out accumulates [q, 65] in
                    # region (h, t3) of the block's single ctx bank.
                    for j in range(3):
                        kt = g * 3 + j
                        for t3 in range(3):
                            r = h * 3 + t3
                            nc.tensor.matmul(
                                cx[:, r * 65:(r + 1) * 65],
                                pt[:, j * QW + t3 * 128:
                                   j * QW + (t3 + 1) * 128],
                                vsb[:, kt * VTW + 65 * h:
                                    kt * VTW + 65 * h + 65],
                                start=(g == 0 and j == 0 and h == 0
                                       and t3 == 0),
                                stop=(g == NG - 1 and j == 2 and h == 1
                                      and t3 == 2),
                                skip_group_check=True)

                def epilogue(qb, cx):
                    for t3 in range(3):
                        t = qb * 3 + t3
                        for h in range(2):
                            r = h * 3 + t3
                            rec = rpool.tile([128, 1], F32, tag="rec",
                                             name=f"rec{qb}_{r}")
                            nc.vector.reciprocal(
                                rec[:], cx[:, r * 65 + 64:r * 65 + 65])
                            nc.vector.tensor_scalar_mul(
                                osb[:, t * 128 + h * 64:t * 128 + h * 64 + 64],
                                cx[:, r * 65:r * 65 + 64], rec[:])
                        # split across partitions: two queues drain in parallel
                        nc.sync.dma_start(
                            out[t * 128:t * 128 + 64, :],
                            osb[0:64, t * 128:(t + 1) * 128])
                        nc.sync.dma_start(
                            out[t * 128 + 64:(t + 1) * 128, :],
                            osb[64:128, t * 128:(t + 1) * 128])

                early = []
                # projections: pq and pk share one PSUM bank -- the pv
                # matmuls emitted between them cover pq's copy drain
                with tc.tile_pool(name="ppsum", bufs=1,
                                  space="PSUM") as ppsum:
                    def proj(cb):
                        c0 = cb * 512
                        pq = ppsum.tile([128, 512], F32, tag="pqk",
                                        name=f"pq{cb}")
                        for j in range(8):
                            nc.tensor.matmul(
                                pq[:], wt[0][j][:],
                                xsb[:, j * S + c0:j * S + c0 + 512],
                                start=(j == 0), stop=(j == 7))
                        nc.vector.tensor_scalar_add(
                            qz[0:64, c0:c0 + 512], pq[0:64, :],
                            bsb[0:64, 0:1])
                        nc.vector.tensor_scalar_add(
                            qz[64:128, S + c0:S + c0 + 512], pq[64:128, :],
                            bsb[64:128, 0:1])
                        pv = ppsum.tile([128, 512], F32, tag="pvv",
                                        name=f"pv{cb}")
                        for kk in range(4):
                            for j in range(8):
                                nc.tensor.matmul(
                                    pv[:, kk * 128:(kk + 1) * 128],
                                    xsb[:, j * S + c0 + kk * 128:
                                        j * S + c0 + (kk + 1) * 128],
                                    wt[2][j][:],
                                    start=(j == 0), stop=(j == 7))
                        pv3 = pv.rearrange("p (k y) -> p k y", y=128)
                        vd = vsb3[:, cb * 4:(cb + 1) * 4, :]
                        nc.scalar.copy(vd[:, :, 0:64], pv3[:, :, 0:64])
                        nc.scalar.copy(vd[:, :, 65:129], pv3[:, :, 64:128])
                        pk = ppsum.tile([128, 512], F32, tag="pqk",
                                        name=f"pk{cb}")
                        for j in range(8):
                            nc.tensor.matmul(
                                pk[:], wt[1][j][:],
                                xsb[:, j * S + c0:j * S + c0 + 512],
                                start=(j == 0), stop=(j == 7))
                        nc.scalar.activation(ksb[:, c0:c0 + 512], pk[:],
                                             AF.Identity, bias=bsb[:, 1:2])

                    # earliest cb whose projections cover each group's keys
                    # (cb_i holds kt 4i..4i+3, g_j needs kt 3j..3j+2)
                    sched = {0: (0,), 1: (1,), 2: (2, 3), 3: (4,),
                             4: (5,), 5: (6, 7)}
                    for cb in range(CB):
                        proj(cb)
                        for g in sched[cb]:
                            early.append((g,) + attn_group(0, g))

                with tc.tile_pool(name="cpsum", bufs=2,
                                  space="PSUM") as cpsum:
                    # catch-up: qb0's deferred P@V pile interleaves with
                    # qb1's first score/exp groups so ScalarE stays fed
                    cx = cpsum.tile([128, 512], F32, tag="ctx", name="cx0")
                    q1g = []
                    for k in range(4):
                        q1g.append(attn_group(1, k))
                        for g, ptE, ptO in early[2 * k:2 * k + 2]:
                            pv_block(cx, ptE, 0, g)
                            pv_block(cx, ptO, 1, g)
                    epilogue(0, cx)

                    cx = cpsum.tile([128, 512], F32, tag="ctx", name="cx1")
                    for g in range(NG):
                        ptE, ptO = q1g[g] if g < 4 else attn_group(1, g)
                        pv_block(cx, ptE, 0, g)
                        pv_block(cx, ptO, 1, g)
                    epilogue(1, cx)

                    for qb in range(2, QB):
                        cx = cpsum.tile([128, 512], F32, tag="ctx",
                                        name=f"cx{qb}")
                        for g in range(NG):
                            ptE, ptO = attn_group(qb, g)
                            pv_block(cx, ptE, 0, g)
                            pv_block(cx, ptO, 1, g)
                        epilogue(qb, cx)

    nc.compile()
    return nc


def _get_program(with_mask: bool):
    key = ("prog", with_mask)
    if key not in _cache:
        _cache[key] = _build(with_mask)
    return _cache[key]


def kernel(hidden_states, attention_mask, Wq, bq, Wk, bk, Wv, bv):
    x = np.asarray(hidden_states, np.float32).reshape(S, HID)
    mask = np.asarray(attention_mask, np.float32).reshape(-1)
    if mask.size == 1:
        mask = np.full(S, float(mask[0]), np.float32)
    with_mask = bool(np.any(mask))

    # transposed weights [3, 1024, 1024]; biases ride separately
    w_all = np.stack([np.asarray(Wq, np.float32).T,
                      np.asarray(Wk, np.float32).T,
                      np.asarray(Wv, np.float32).T]).astype(ml_dtypes.bfloat16)
    bq = np.asarray(bq, np.float32)
    bk = np.asarray(bk, np.float32)
    bv = np.asarray(bv, np.float32)

    xtc = np.ascontiguousarray(x.T).astype(ml_dtypes.bfloat16)
    if with_mask:
        maskt = np.ascontiguousarray(
            mask.reshape(KT, 128).T.astype(np.float32))

    nc = _get_program(with_mask)
    in_maps = []
    for c in range(N_CORES):
        sl = slice(c * 128, (c + 1) * 128)
        m = {
            "xt": xtc,
            "w": np.ascontiguousarray(w_all[:, :, sl]),
            "bcol": np.ascontiguousarray(
                np.stack([bq[sl], bk[sl]], axis=1)),
        }
        if with_mask:
            m["maskt"] = maskt
        in_maps.append(m)

    _cache["last_in_maps"] = in_maps
    res = bass_utils.run_bass_kernel_spmd(nc, in_maps, core_ids=list(range(N_CORES)))
    out = np.concatenate([res.results[c]["out"] for c in range(N_CORES)], axis=1)
    out = out + bv[None, :]
    return out.reshape(B, S, HID).astype(np.float32)
